# revision 47
# baseline (speedup 1.0000x reference)
"""Trainium2 Bass kernel: channel-attention MultiHeadAttention block.

Full (unsharded) inputs in, full output out. Data-parallel over batch B
across 8 NeuronCores (1 batch each), tiny AllReduce for BatchNorm stats.

Algorithmic structure (per core, batch b), exploiting the 1x1-conv
low-rank structure of the attention:

  scores = (Wq q)(Wk k)^T = Wq (q k^T) Wk^T   with  G = q k^T  [65,65]
  out    = softmax(scores) (Wv v) = H v        with  H = attn Wv [512,65]

Everything is computed in TRANSPOSED orientation (ST[d,c]) so that the
softmax normalizer and the BatchNorm statistics can be derived from
small factored matrices *before* the big X tensor is materialized:

  sum_j X[a,j]   = sum_t W[t,a] u[t]          W = fold_r(v), u = HTn 1
  sum_j X[a,j]^2 = sum_r diag(v_r^T M v_r)    M = HTn HTn^T  [65,65]

so the BN AllReduce is fired early and overlaps X production.
All heavy matmuls run in bf16 (1 cyc/row, FWL weight loads, no
fp32_mode=HIGH power throttle).
"""

import sys

if "/opt/trn_rl_repo" not in sys.path:
    sys.path.insert(0, "/opt/trn_rl_repo")

import ml_dtypes
import numpy as np

import concourse.bacc as bacc
import concourse.mybir as mybir
import concourse.tile as tile
from concourse import bass_utils

B = 8
C = 64
CN = 512
HW = 4096
NCH = 4         # 128-chunks of the 512 dims
NS = 8          # 512-wide spatial slices
NMC = 32        # 128-wide m-chunks
EPS = 1e-4
SLOPE = 0.01
INV_SCALE = 1.0 / 64.0
INV_BHW = 1.0 / (B * HW)

F32 = mybir.dt.float32
F32R = mybir.dt.float32r
BF16 = mybir.dt.bfloat16
AF = mybir.ActivationFunctionType
ALU = mybir.AluOpType
AX = mybir.AxisListType
RG = [[0, 1, 2, 3, 4, 5, 6, 7]]

# wpack (bf16) column layout: early part (score chain) then late (w1/w2)
WQE0 = 0
WKE0 = 512
WVT0 = 1024
IDB0 = 1284
WEARLY = IDB0 + 128         # 1412
W1T0 = WEARLY
W2T0 = W1T0 + 2048          # 3460
WCOLS = W2T0 + 256          # 3716


def _r(ap):
    return ap.bitcast(F32R)


def _body(tc, nc, d, dbg=None):
    with (
        tc.tile_pool(name="consts", bufs=1) as consts,
        tc.tile_pool(name="small", bufs=1) as small,
        tc.tile_pool(name="xbuf", bufs=1) as xpool,
    ):
        # ---- weights / constants
        wpack = consts.tile([128, WCOLS], BF16, name="wpack", tag="wpack")
        spack = consts.tile([128, 16], F32, name="spack", tag="spack")
        wqe = wpack[0:65, WQE0:WQE0 + 512]
        wke = wpack[0:65, WKE0:WKE0 + 512]
        b1sb = spack[:, 0:4]
        bngsb = spack[:, 4:8]
        bnbsb = spack[:, 8:12]
        b2sb = spack[0:64, 12:13]
        identb = wpack[:, IDB0:IDB0 + 128]

        # ---- inputs
        qt = consts.tile([128, 2080], BF16, name="qt", tag="qt")
        kt = consts.tile([128, 2080], BF16, name="kt", tag="kt")
        vt = consts.tile([65, HW], BF16, name="vt", tag="vt")
        # halved q/k transfers first so the gram matmuls chase the DMA;
        # weights follow (wqe needed only once the gram completes)
        nc.sync.dma_start(qt[:, 0:1040], d["qt"][:, 0:1040])
        nc.scalar.dma_start(kt[:, 0:1040], d["kt"][:, 0:1040])
        nc.sync.dma_start(qt[:, 1040:2080], d["qt"][:, 1040:2080])
        nc.scalar.dma_start(kt[:, 1040:2080], d["kt"][:, 1040:2080])
        nc.sync.dma_start(wpack[:, 0:WEARLY], d["wpack"][:, 0:WEARLY])
        nc.scalar.dma_start(vt[:, 0:2048], d["vt"][:, 0:2048])
        nc.scalar.dma_start(vt[:, 2048:4096], d["vt"][:, 2048:4096])
        nc.sync.dma_start(wpack[:, WEARLY:WCOLS],
                          d["wpack"][:, WEARLY:WCOLS])
        nc.scalar.dma_start(spack[:], d["spack"][:])

        # PE warmup spin during the input-DMA wait: ~5us of junk matmuls
        # flips the HAM clock gate to K=8/8 before the gram starts, and the
        # front's inter-op gaps stay under the ~3.4us re-throttle window.
        with (
            tc.tile_pool(name="wrm", bufs=1) as wrm,
            tc.tile_pool(name="wrmps", bufs=1, space="PSUM") as wrmps,
        ):
            wsp = wrm.tile([128, 512], BF16, name="wsp", tag="wsp")
            nc.gpsimd.memset(wsp[:, 0:1], 1.0)
            wps_ = wrmps.tile([128, 512], F32, name="wps_", tag="wps_")
            for i in range(24):
                nc.tensor.matmul(wps_[:], wsp[:, 0:128], wsp[:],
                                 start=True, stop=True)

        ones65 = small.tile([65, 1], BF16, name="ones65", tag="ones65")
        nc.gpsimd.memset(ones65[:], 1.0)
        epsb = small.tile([128, 1], F32, name="epsb", tag="epsb")
        nc.gpsimd.memset(epsb[:], EPS)
        ones128 = small.tile([128, 1], BF16, name="ones128", tag="ones128")
        nc.gpsimd.memset(ones128[:], 1.0)
        # preload the Exp ACT table during the DMA wait: otherwise the
        # ~2.7us table load lands on the score-chain critical path
        expwarm = small.tile([128, 1], F32, name="expwarm", tag="expwarm")
        nc.scalar.activation(expwarm[:], epsb[:], AF.Exp, bias=0.0,
                             scale=1.0)

        X = [xpool.tile([128, HW], BF16, name=f"X{cc}", tag=f"X{cc}")
             for cc in range(NCH)]
        HTn = small.tile([65, 512], BF16, name="HTn", tag="HTn")
        alpha = small.tile([128, 4], F32, name="alpha", tag="alpha")
        beta = small.tile([128, 4], F32, name="beta", tag="beta")

        # W = fold_r(v): W[t,a] = sum_r v~[t, 8a+r] = sum of the 8 r-blocks
        Wf = small.tile([65, 512], F32, name="Wf", tag="Wf")
        nc.vector.tensor_copy(Wf[:], vt[:, 0:512])
        for r in range(1, NS):
            nc.vector.tensor_add(Wf[:], Wf[:], vt[:, 512 * r:512 * (r + 1)])

        # ============ phase 1: gram + score chain + softmax(T) ==========
        with tc.tile_pool(name="ph1", bufs=1) as ph1:
            with tc.tile_pool(name="gps", bufs=1, space="PSUM") as gps:
                # G[t,j] = sum_m qT[m,t] kT[m,j], accumulated over m-chunks
                g_ps = gps.tile([65, 65], F32, name="g_ps", tag="g_ps")
                for i in range(NMC):
                    nc.tensor.matmul(g_ps[:], qt[:, 65 * i:65 * (i + 1)],
                                     kt[:, 65 * i:65 * (i + 1)],
                                     start=(i == 0), stop=(i == NMC - 1))
                g_sb = ph1.tile([65, 65], BF16, name="g_sb", tag="g_sb")
                nc.scalar.copy(g_sb[:], g_ps[:])

                # Pp = G^T Wq~^T : Pp[j,c] = sum_t G[t,j] wqe[t,c]
                pp_ps = gps.tile([65, 512], F32, name="pp_ps", tag="pp_ps")
                nc.tensor.matmul(pp_ps[:], g_sb[:], wqe, start=True,
                                 stop=True)
                pp_sb = ph1.tile([65, 512], BF16, name="pp_sb", tag="pp_sb")
                nc.scalar.copy(pp_sb[:], pp_ps[:])

            ET = [ph1.tile([128, 512], BF16, name=f"ET{dc}", tag=f"ET{dc}")
                  for dc in range(NCH)]
            with tc.tile_pool(name="stps", bufs=1, space="PSUM") as stps:
                # ST[d,c] = sum_j wke[j,d] Pp[j,c]  (4 d-chunks)
                st_ps = [stps.tile([128, 512], F32, name=f"st{dc}",
                                   tag=f"st{dc}") for dc in range(NCH)]
                # rowsums land transposed: column 4*dc+j of rsp_ps, all
                # single-shot matmuls (no in-bank accumulation groups)
                rsp_ps = stps.tile([128, 16], F32, name="rsp_ps",
                                   tag="rsp_ps")
                for dc in range(NCH):
                    nc.tensor.matmul(st_ps[dc][:],
                                     wke[:, 128 * dc:128 * (dc + 1)],
                                     pp_sb[:], start=True, stop=True)
                    nc.scalar.activation(ET[dc][:], st_ps[dc][:], AF.Exp,
                                         bias=0.0, scale=INV_SCALE)
                    for j in range(NCH):
                        nc.tensor.matmul(
                            rsp_ps[:, 4 * dc + j:4 * dc + j + 1],
                            ET[dc][:, 128 * j:128 * (j + 1)],
                            ones128[:], start=True, stop=True)
                rsum = ph1.tile([128, 4], F32, name="rsum", tag="rsum")
                nc.vector.reduce_sum(
                    rsum[:], rsp_ps.rearrange("p (dc j) -> p j dc", j=4),
                    axis=AX.X)
                recip_p = small.tile([128, 4], F32, name="recip_p",
                                     tag="recip_p")
                nc.vector.reciprocal(recip_p[:], rsum[:])

            with tc.tile_pool(name="hps", bufs=1, space="PSUM") as hps:
                # HTu = Wv~^T E^T : 4 accumulating matmuls
                htu_ps = hps.tile([65, 512], F32, name="htu_ps",
                                  tag="htu_ps")
                for dc in range(NCH):
                    nc.tensor.matmul(
                        htu_ps[:],
                        wpack[:, WVT0 + 65 * dc:WVT0 + 65 * (dc + 1)],
                        ET[dc][:], start=(dc == 0), stop=(dc == NCH - 1))
                htu_sb = ph1.tile([65, 512], BF16, name="htu_sb",
                                  tag="htu_sb")
                nc.scalar.copy(htu_sb[:], htu_ps[:])

                # H_n1[c, t] = HTu^T * recip (transpose + per-part scale)
                # (66-col stride: PSUM writes must be 4B aligned)
                tp_ps = hps.tile([128, 264], BF16, name="tp_ps", tag="tp_ps")
                h_n1 = ph1.tile([128, 260], BF16, name="h_n1", tag="h_n1")
                for j in range(NCH):
                    nc.tensor.transpose(tp_ps[:, 66 * j:66 * j + 65],
                                        htu_sb[:, 128 * j:128 * (j + 1)],
                                        identb[0:65, 0:65])
                    nc.scalar.activation(h_n1[:, 65 * j:65 * (j + 1)],
                                         tp_ps[:, 66 * j:66 * j + 65],
                                         AF.Copy, bias=0.0,
                                         scale=recip_p[:, j:j + 1])
                # M[t,t'] = sum_c Hn1[c,t] Hn1[c,t']
                # (full-bank tile: accumulation groups must own their bank —
                # any start=True matmul clears has_written for the whole bank)
                m_ps = hps.tile([65, 512], F32, name="m_ps", tag="m_ps")
                for j in range(NCH):
                    nc.tensor.matmul(m_ps[:, 0:65],
                                     h_n1[:, 65 * j:65 * (j + 1)],
                                     h_n1[:, 65 * j:65 * (j + 1)],
                                     start=(j == 0), stop=(j == NCH - 1))
                m_sb = small.tile([65, 65], BF16, name="m_sb", tag="m_sb")
                nc.vector.tensor_copy(m_sb[:], m_ps[:, 0:65])
                # HTn = transpose back: [65, 512]
                btp_ps = hps.tile([65, 512], BF16, name="btp_ps",
                                  tag="btp_ps")
                for j in range(NCH):
                    nc.tensor.transpose(btp_ps[:, 128 * j:128 * (j + 1)],
                                        h_n1[:, 65 * j:65 * (j + 1)],
                                        identb[:, 0:128])
                nc.scalar.copy(HTn[:], btp_ps[:])
            # u = HTn @ 1 : row sums [65, 1]
            u_sb = small.tile([65, 1], F32, name="u_sb", tag="u_sb")
            nc.vector.reduce_sum(u_sb[:], HTn[:], axis=AX.X)

            if dbg is not None:
                nc.sync.dma_start(dbg["g"][:], g_sb[:])
                for dc in range(NCH):
                    nc.sync.dma_start(dbg[f"ET{dc}"][:], ET[dc][:])
                nc.sync.dma_start(dbg["HTn"][:], HTn[:])
                nc.sync.dma_start(dbg["m"][:], m_sb[:])

        # ============ phase 2: factored BN stats -> AllReduce ===========
        # (emitted before X so the PE prioritizes unblocking the AR)
        with (
            tc.tile_pool(name="ph2", bufs=1) as ph2,
            tc.tile_pool(name="ssps", bufs=1, space="PSUM") as ssps,
            tc.tile_pool(name="zps", bufs=2, space="PSUM") as zps,
            tc.tile_pool(name="cdram", bufs=1, space="DRAM") as cdram,
            tc.tile_pool(name="xps", bufs=3, space="PSUM") as xps,
            tc.tile_pool(name="stp", bufs=1) as stp,
        ):
            # stat matmuls are all single-shot writes to distinct columns of
            # one bank (no PSUM accumulation groups — a start=True clears
            # has_written bank-wide); the r-reduction happens on the DVE.
            ss_ps = ssps.tile([128, 36], F32, name="ss_ps", tag="ss_ps")
            # sum[a] = sum_t W[t,a] u[t]
            for cc in range(NCH):
                nc.tensor.matmul(ss_ps[:, cc:cc + 1],
                                 Wf[:, 128 * cc:128 * (cc + 1)],
                                 u_sb[:], start=True, stop=True)
            # P[t, 512r + a] = v_r[t,a] * (M v_r)[t,a]; r-pairs share one
            # [65,1024] psum tile so each DVE multiply covers two r at once
            # (fewer DVE drains + semaphore hops on the AR critical path)
            P = ph2.tile([65, HW], BF16, name="P", tag="P")
            for g in range(NS // 2):
                z_ps = zps.tile([65, 1024], F32, name="z_ps", tag="z_ps")
                for h in range(2):
                    r = 2 * g + h
                    nc.tensor.matmul(z_ps[:, 512 * h:512 * (h + 1)],
                                     m_sb[:], vt[:, 512 * r:512 * (r + 1)],
                                     start=True, stop=True)
                nc.vector.tensor_mul(
                    P[:, 1024 * g:1024 * (g + 1)],
                    vt[:, 1024 * g:1024 * (g + 1)], z_ps[:])
            # sumsq part (cc, r) -> column 4 + 8*cc + r
            for r in range(NS):
                for cc in range(NCH):
                    nc.tensor.matmul(
                        ss_ps[:, 4 + 8 * cc + r:5 + 8 * cc + r],
                        P[:, 512 * r + 128 * cc:512 * r + 128 * (cc + 1)],
                        ones65[:], start=True, stop=True)
            red = stp.tile([128, 8], F32, name="red", tag="red")
            nc.vector.tensor_copy(red[:, 0:4], ss_ps[:, 0:4])
            nc.vector.reduce_sum(
                red[:, 4:8],
                ss_ps[:, 4:36].rearrange("p (cc r) -> p cc r", r=8),
                axis=AX.X)

            cin = cdram.tile([128, 8], F32, name="cin", tag="cin")
            cout = cdram.tile([128, 8], F32, name="cout", tag="cout")
            nc.gpsimd.dma_start(cin[:], red[:])
            nc.gpsimd.collective_compute(
                "AllReduce", ALU.add, replica_groups=RG,
                ins=[cin.opt()], outs=[cout.opt()])
            ar = stp.tile([128, 8], F32, name="ar", tag="ar")
            nc.sync.dma_start(ar[:], cout[:])

            # ======== phase 3: X = v_r^T HTn (overlaps the AllReduce) ====
            for cc in range(NCH):
                for r in range(NS):
                    xt = xps.tile([128, 512], F32, name="xt", tag="xt")
                    nc.tensor.matmul(
                        xt[:],
                        vt[:, 512 * r + 128 * cc:512 * r + 128 * (cc + 1)],
                        HTn[:], start=True, stop=True)
                    xsl = slice(512 * r, 512 * (r + 1))
                    if r % 2 == 0:
                        nc.scalar.copy(X[cc][:, xsl], xt[:])
                    else:
                        nc.vector.tensor_copy(X[cc][:, xsl], xt[:])

            # preload ACT tables while the AllReduce is in flight; the
            # read of `red` pins these after the stats trigger so their
            # table loads cannot be hoisted into the score chain
            dummy = stp.tile([128, 1], F32, name="dummy", tag="dummy")
            nc.scalar.activation(dummy[:], red[:, 0:1], AF.Lrelu,
                                 bias=0.0, scale=1.0, alpha=SLOPE)
            nc.scalar.activation(dummy[:], red[:, 0:1], AF.Sqrt,
                                 bias=epsb[:, 0:1])

            # BN affine params
            sd = stp.tile([128, 4], F32, name="sd", tag="sd")
            rstd = stp.tile([128, 4], F32, name="rstd", tag="rstd")
            tmp = stp.tile([128, 4], F32, name="tmpb", tag="tmpb")
            mv = stp.tile([128, 8], F32, name="mv", tag="mv")
            nc.vector.tensor_scalar_mul(mv[:], ar[:], INV_BHW)
            mean = mv[:, 0:4]
            var = mv[:, 4:8]
            nc.vector.tensor_mul(tmp[:], mean, mean)
            nc.vector.tensor_sub(var, var, tmp[:])
            nc.scalar.activation(sd[:], var, AF.Sqrt, bias=epsb[:, 0:1])
            nc.vector.reciprocal(rstd[:], sd[:])
            nc.vector.tensor_mul(alpha[:], bngsb, rstd[:])
            nc.vector.tensor_mul(tmp[:], mean, alpha[:])
            nc.vector.tensor_sub(beta[:], bnbsb, tmp[:])

            if dbg is not None:
                nc.sync.dma_start(dbg["P"][:], P[:])
                nc.sync.dma_start(dbg["red"][:], red[:])
                for cc in range(NCH):
                    nc.sync.dma_start(dbg[f"X{cc}"][:], X[cc][:])
                    nc.sync.dma_start(dbg[f"ar{cc}"][:, 0:1],
                                      ar[:, cc:cc + 1])
                    nc.sync.dma_start(dbg[f"ar{cc}"][:, 1:2],
                                      ar[:, 4 + cc:5 + cc])

        # ============ phase 4: BN+leaky -> w1 -> leaky -> w2 -> y =======
        with (
            tc.tile_pool(name="y2", bufs=2) as y2p,
            tc.tile_pool(name="bnp", bufs=2) as bnp,
            tc.tile_pool(name="outb", bufs=1) as outp,
            tc.tile_pool(name="wps", bufs=3, space="PSUM") as wps,
            tc.tile_pool(name="w2ps", bufs=2, space="PSUM") as w2ps,
        ):
            osb = outp.tile([64, HW], F32, name="osb", tag="osb")

            def bn_act(ms):
                # first slice on ACT (idle right after the AR): single-op
                # Lrelu unblocks the first w1 matmuls ~0.7us sooner and runs
                # in parallel with the DVE's alpha/beta + bn(1)
                ssl = slice(512 * ms, 512 * (ms + 1))
                for cc in range(NCH):
                    nc.scalar.activation(X[cc][:, ssl], X[cc][:, ssl],
                                         AF.Lrelu, bias=beta[:, cc:cc + 1],
                                         scale=alpha[:, cc:cc + 1],
                                         alpha=SLOPE)

            def bn(ms, width=1):
                # on DVE (ACT is saturated by y2): t = a*X+b; X = max(t, .01t)
                ssl = slice(512 * ms, 512 * (ms + width))
                for cc in range(NCH):
                    t = bnp.tile([128, 512 * width], BF16, name="bnt",
                                 tag="bnt", bufs=2)
                    nc.vector.tensor_scalar(
                        out=t[:], in0=X[cc][:, ssl],
                        scalar1=alpha[:, cc:cc + 1],
                        scalar2=beta[:, cc:cc + 1],
                        op0=ALU.mult, op1=ALU.add)
                    nc.vector.scalar_tensor_tensor(
                        out=X[cc][:, ssl], in0=t[:], scalar=SLOPE,
                        in1=t[:], op0=ALU.mult, op1=ALU.max)

            def tail_pair(mp):
                # two ms-slices (2mp, 2mp+1) share [128,1024] psum tiles so
                # the Lrelu activations run 1024 wide (amortize ACT overhead)
                y2t = []
                for oc in range(NCH):
                    wp = wps.tile([128, 1024], F32, name="wp", tag="wp")
                    for h in range(2):
                        ssl = slice(512 * (2 * mp + h), 512 * (2 * mp + h + 1))
                        for cc in range(NCH):
                            nc.tensor.matmul(
                                wp[:, 512 * h:512 * (h + 1)],
                                wpack[:, W1T0 + 512 * cc + 128 * oc:
                                      W1T0 + 512 * cc + 128 * (oc + 1)],
                                X[cc][:, ssl], start=(cc == 0), stop=(cc == 3))
                    yt = y2p.tile([128, 1024], BF16, name=f"y2_{oc}",
                                  tag=f"y2_{oc}")
                    nc.scalar.activation(yt[:], wp[:], AF.Lrelu,
                                         bias=b1sb[:, oc:oc + 1],
                                         scale=1.0, alpha=SLOPE)
                    y2t.append(yt)
                for h in range(2):
                    ssl = slice(512 * (2 * mp + h), 512 * (2 * mp + h + 1))
                    fp = w2ps.tile([64, 512], F32, name="fp", tag="fp")
                    for oc in range(NCH):
                        nc.tensor.matmul(
                            fp[:],
                            wpack[:, W2T0 + 64 * oc:W2T0 + 64 * (oc + 1)],
                            y2t[oc][:, 512 * h:512 * (h + 1)],
                            start=(oc == 0), stop=(oc == 3))
                    nc.vector.tensor_scalar_add(osb[:, ssl], fp[:],
                                                b2sb[:, 0:1])
                    if h == 0:
                        nc.sync.dma_start(d["y"][:, ssl], osb[:, ssl])
                    else:
                        nc.scalar.dma_start(d["y"][:, ssl], osb[:, ssl])

            def tail_one(ms):
                ssl = slice(512 * ms, 512 * (ms + 1))
                y2t = []
                for oc in range(NCH):
                    wp = wps.tile([128, 1024], F32, name="wp", tag="wp")
                    for cc in range(NCH):
                        nc.tensor.matmul(
                            wp[:, 0:512],
                            wpack[:, W1T0 + 512 * cc + 128 * oc:
                                  W1T0 + 512 * cc + 128 * (oc + 1)],
                            X[cc][:, ssl], start=(cc == 0), stop=(cc == 3))
                    yt = y2p.tile([128, 1024], BF16, name=f"y2_{oc}",
                                  tag=f"y2_{oc}")
                    nc.scalar.activation(yt[:, 0:512], wp[:, 0:512],
                                         AF.Lrelu, bias=b1sb[:, oc:oc + 1],
                                         scale=1.0, alpha=SLOPE)
                    y2t.append(yt)
                fp = w2ps.tile([64, 512], F32, name="fp", tag="fp")
                for oc in range(NCH):
                    nc.tensor.matmul(
                        fp[:], wpack[:, W2T0 + 64 * oc:W2T0 + 64 * (oc + 1)],
                        y2t[oc][:, 0:512], start=(oc == 0), stop=(oc == 3))
                nc.vector.tensor_scalar_add(osb[:, ssl], fp[:], b2sb[:, 0:1])
                nc.sync.dma_start(d["y"][:, ssl], osb[:, ssl])

            bn(0)
            bn(1)
            for mp in range(3):
                tail_pair(mp)
                bn(2 * mp + 2, width=2)
            tail_one(6)
            tail_one(7)


_NC_CACHE = {}


def _build(debug=False):
    key = ("dbg" if debug else "nc")
    if key in _NC_CACHE:
        return _NC_CACHE[key]
    nc = bacc.Bacc(trn_type="TRN2", target_bir_lowering=False, debug=False,
                   enable_asserts=False, num_devices=8)
    d = {}
    d["qt"] = nc.dram_tensor("qt", (128, 2080), BF16, kind="ExternalInput").ap()
    d["kt"] = nc.dram_tensor("kt", (128, 2080), BF16, kind="ExternalInput").ap()
    d["vt"] = nc.dram_tensor("vt", (65, HW), BF16, kind="ExternalInput").ap()
    d["wpack"] = nc.dram_tensor("wpack", (128, WCOLS), BF16,
                                kind="ExternalInput").ap()
    d["spack"] = nc.dram_tensor("spack", (128, 16), F32,
                                kind="ExternalInput").ap()
    d["y"] = nc.dram_tensor("y", (64, HW), F32, kind="ExternalOutput").ap()

    dbg = None
    if debug:
        dbg = {}
        dbg["g"] = nc.dram_tensor("dbg_g", (65, 65), BF16, kind="ExternalOutput").ap()
        dbg["HTn"] = nc.dram_tensor("dbg_HTn", (65, 512), BF16, kind="ExternalOutput").ap()
        dbg["m"] = nc.dram_tensor("dbg_m", (65, 65), BF16, kind="ExternalOutput").ap()
        dbg["P"] = nc.dram_tensor("dbg_P", (65, HW), BF16, kind="ExternalOutput").ap()
        dbg["red"] = nc.dram_tensor("dbg_red", (128, 8), F32, kind="ExternalOutput").ap()
        for cc in range(NCH):
            dbg[f"ET{cc}"] = nc.dram_tensor(f"dbg_ET{cc}", (128, 512), BF16, kind="ExternalOutput").ap()
            dbg[f"X{cc}"] = nc.dram_tensor(f"dbg_X{cc}", (128, HW), BF16, kind="ExternalOutput").ap()
            dbg[f"ar{cc}"] = nc.dram_tensor(f"dbg_ar{cc}", (128, 2), F32, kind="ExternalOutput").ap()
    with tile.TileContext(nc) as tc:
        _body(tc, nc, d, dbg)
    nc.compile()
    _NC_CACHE[key] = nc
    return nc


def _prep(q, k, v, wq, bq, wk, bk, wv, bv, bn_g, bn_b, w1, b1, w2, b2):
    f = np.float32
    bf = ml_dtypes.bfloat16
    wpack = np.zeros((128, WCOLS), f)
    wpack[0:65, WQE0:WQE0 + 512] = np.concatenate([wq.T, bq[None, :]], axis=0)
    wpack[0:65, WKE0:WKE0 + 512] = np.concatenate([wk.T, bk[None, :]], axis=0)
    wve = np.concatenate([wv.T, bv[None, :]], axis=0)  # [65, 512] = Wv^T aug
    for j in range(NCH):
        # wvT chunk j: [128d, 65t] = Wv~[128j:128(j+1), :] = wve.T slice
        wpack[:, WVT0 + 65 * j:WVT0 + 65 * (j + 1)] = \
            wve[:, 128 * j:128 * (j + 1)].T
    w1t = w1.T.astype(f)
    for cc in range(NCH):
        wpack[:, W1T0 + 512 * cc:W1T0 + 512 * (cc + 1)] = \
            w1t[128 * cc:128 * (cc + 1), :]
    w2t = w2.T.astype(f)
    for oc in range(NCH):
        wpack[:, W2T0 + 64 * oc:W2T0 + 64 * (oc + 1)] = \
            w2t[128 * oc:128 * (oc + 1), :]
    wpack[:, IDB0:IDB0 + 128] = np.eye(128, dtype=f)
    spack = np.zeros((128, 16), f)
    spack[:, 0:4] = b1.reshape(4, 128).T
    spack[:, 4:8] = bn_g.reshape(4, 128).T
    spack[:, 8:12] = bn_b.reshape(4, 128).T
    spack[0:64, 12] = b2

    shared = {"wpack": wpack.astype(bf), "spack": spack}
    in_maps = []
    ones_col = np.ones((HW, 1), f)
    for b in range(B):
        m = dict(shared)
        for name, x in (("qt", q[b]), ("kt", k[b])):
            xt = np.concatenate([x.reshape(C, HW).T, ones_col], axis=1)
            m[name] = np.ascontiguousarray(
                xt.reshape(NMC, 128, 65).transpose(1, 0, 2).reshape(128, 2080)
            ).astype(bf)
        vtn = np.concatenate(
            [v[b].reshape(C, HW), np.ones((1, HW), f)], axis=0)
        # permute to r-major blocks: vt[t, 512r + a] = v~[t, 8a + r], so
        # every device-side slice of v is contiguous (stride-8 access
        # patterns run the PE/DVE at 1/3-1/5 rate)
        m["vt"] = np.ascontiguousarray(
            vtn.reshape(65, 512, 8).transpose(0, 2, 1).reshape(65, HW)
        ).astype(bf)
        in_maps.append(m)
    return in_maps


def _run(q, k, v, wq, bq, wk, bk, wv, bv, bn_g, bn_b, w1, b1, w2, b2,
         trace=False, tmpdir=None, debug=False):
    nc = _build(debug)
    in_maps = _prep(q, k, v, wq, bq, wk, bk, wv, bv, bn_g, bn_b, w1, b1,
                    w2, b2)
    res = bass_utils.run_bass_kernel_spmd(
        nc, in_maps, core_ids=list(range(8)), trace=trace, tmpdir=tmpdir)
    out = np.stack([res.results[b]["y"].reshape(C, 64, 64) for b in range(B)])
    return out.astype(np.float32), res


def kernel(q, k, v, wq, bq, wk, bk, wv, bv, bn_g, bn_b, w1, b1, w2, b2):
    out, _ = _run(q, k, v, wq, bq, wk, bk, wv, bv, bn_g, bn_b, w1, b1, w2, b2)
    return out


# revision 48
# speedup vs baseline: 1.0811x; 1.0811x over previous
"""Trainium2 Bass kernel: channel-attention MultiHeadAttention block.

Full (unsharded) inputs in, full output out. Data-parallel over batch B
across 8 NeuronCores (1 batch each), tiny AllReduce for BatchNorm stats.

Algorithmic structure (per core, batch b), exploiting the 1x1-conv
low-rank structure of the attention:

  scores = (Wq q)(Wk k)^T = Wq (q k^T) Wk^T   with  G = q k^T  [65,65]
  out    = softmax(scores) (Wv v) = H v        with  H = attn Wv [512,65]

Everything is computed in TRANSPOSED orientation (ST[d,c]) so that the
softmax normalizer and the BatchNorm statistics can be derived from
small factored matrices *before* the big X tensor is materialized:

  sum_j X[a,j]   = sum_t W[t,a] u[t]          W = fold_r(v), u = HTn 1
  sum_j X[a,j]^2 = sum_r diag(v_r^T M v_r)    M = HTn HTn^T  [65,65]

so the BN AllReduce is fired early and overlaps X production.
All heavy matmuls run in bf16 (1 cyc/row, FWL weight loads, no
fp32_mode=HIGH power throttle).
"""

import sys

if "/opt/trn_rl_repo" not in sys.path:
    sys.path.insert(0, "/opt/trn_rl_repo")

import ml_dtypes
import numpy as np

import concourse.bacc as bacc
import concourse.mybir as mybir
import concourse.tile as tile
from concourse import bass_utils

B = 8
C = 64
CN = 512
HW = 4096
NCH = 4         # 128-chunks of the 512 dims
NS = 8          # 512-wide spatial slices
NMC = 32        # 128-wide m-chunks
EPS = 1e-4
SLOPE = 0.01
INV_SCALE = 1.0 / 64.0
INV_BHW = 1.0 / (B * HW)

F32 = mybir.dt.float32
F32R = mybir.dt.float32r
BF16 = mybir.dt.bfloat16
AF = mybir.ActivationFunctionType
ALU = mybir.AluOpType
AX = mybir.AxisListType
RG = [[0, 1, 2, 3, 4, 5, 6, 7]]

# wpack (bf16) column layout: early part (score chain) then late (w1/w2)
WQE0 = 0
WKE0 = 512
WVT0 = 1024
IDB0 = 1284
WEARLY = IDB0 + 128         # 1412
W1T0 = WEARLY
W2T0 = W1T0 + 2048          # 3460
WCOLS = W2T0 + 256          # 3716


def _r(ap):
    return ap.bitcast(F32R)


def _body(tc, nc, d, dbg=None):
    with (
        tc.tile_pool(name="consts", bufs=1) as consts,
        tc.tile_pool(name="small", bufs=1) as small,
        tc.tile_pool(name="xbuf", bufs=1) as xpool,
    ):
        # ---- weights / constants
        wpack = consts.tile([128, WCOLS], BF16, name="wpack", tag="wpack")
        spack = consts.tile([128, 16], F32, name="spack", tag="spack")
        wqe = wpack[0:65, WQE0:WQE0 + 512]
        wke = wpack[0:65, WKE0:WKE0 + 512]
        b1sb = spack[:, 0:4]
        bngsb = spack[:, 4:8]
        bnbsb = spack[:, 8:12]
        b2sb = spack[0:64, 12:13]
        identb = wpack[:, IDB0:IDB0 + 128]

        # ---- inputs
        qt = consts.tile([128, 2080], BF16, name="qt", tag="qt")
        kt = consts.tile([128, 2080], BF16, name="kt", tag="kt")
        vt = consts.tile([65, HW], BF16, name="vt", tag="vt")
        # halved q/k transfers first so the gram matmuls chase the DMA;
        # weights follow (wqe needed only once the gram completes)
        nc.sync.dma_start(qt[:, 0:1040], d["qt"][:, 0:1040])
        nc.scalar.dma_start(kt[:, 0:1040], d["kt"][:, 0:1040])
        nc.sync.dma_start(qt[:, 1040:2080], d["qt"][:, 1040:2080])
        nc.scalar.dma_start(kt[:, 1040:2080], d["kt"][:, 1040:2080])
        nc.sync.dma_start(wpack[:, 0:WEARLY], d["wpack"][:, 0:WEARLY])
        nc.scalar.dma_start(vt[:, 0:2048], d["vt"][:, 0:2048])
        nc.scalar.dma_start(vt[:, 2048:4096], d["vt"][:, 2048:4096])
        nc.sync.dma_start(wpack[:, WEARLY:WCOLS],
                          d["wpack"][:, WEARLY:WCOLS])
        nc.scalar.dma_start(spack[:], d["spack"][:])

        # PE warmup spin during the input-DMA wait: ~5us of junk matmuls
        # flips the HAM clock gate to K=8/8 before the gram starts, and the
        # front's inter-op gaps stay under the ~3.4us re-throttle window.
        with (
            tc.tile_pool(name="wrm", bufs=1) as wrm,
            tc.tile_pool(name="wrmps", bufs=1, space="PSUM") as wrmps,
        ):
            wsp = wrm.tile([128, 512], BF16, name="wsp", tag="wsp")
            nc.gpsimd.memset(wsp[:, 0:1], 1.0)
            wps_ = wrmps.tile([128, 512], F32, name="wps_", tag="wps_")
            # 20 x 427ns cold = ~8.5us continuous busy: >= 2 full HAM
            # windows (flip guaranteed) while ending before the q/k DMA
            # lands, so the spin never blocks the gram in the PE FIFO
            for i in range(20):
                nc.tensor.matmul(wps_[:], wsp[:, 0:128], wsp[:],
                                 start=True, stop=True)

        ones65 = small.tile([65, 1], BF16, name="ones65", tag="ones65")
        nc.gpsimd.memset(ones65[:], 1.0)
        epsb = small.tile([128, 1], F32, name="epsb", tag="epsb")
        nc.gpsimd.memset(epsb[:], EPS)
        ones128 = small.tile([128, 1], BF16, name="ones128", tag="ones128")
        nc.gpsimd.memset(ones128[:], 1.0)
        # preload the Exp ACT table during the DMA wait: otherwise the
        # ~2.7us table load lands on the score-chain critical path
        expwarm = small.tile([128, 1], F32, name="expwarm", tag="expwarm")
        nc.scalar.activation(expwarm[:], epsb[:], AF.Exp, bias=0.0,
                             scale=1.0)

        X = [xpool.tile([128, HW], BF16, name=f"X{cc}", tag=f"X{cc}")
             for cc in range(NCH)]
        HTn = small.tile([65, 512], BF16, name="HTn", tag="HTn")
        alpha = small.tile([128, 4], F32, name="alpha", tag="alpha")
        beta = small.tile([128, 4], F32, name="beta", tag="beta")

        # W = fold_r(v): W[t,a] = sum_r v~[t, 8a+r] = sum of the 8 r-blocks
        Wf = small.tile([65, 512], F32, name="Wf", tag="Wf")
        nc.vector.tensor_copy(Wf[:], vt[:, 0:512])
        for r in range(1, NS):
            nc.vector.tensor_add(Wf[:], Wf[:], vt[:, 512 * r:512 * (r + 1)])

        # ============ phase 1: gram + score chain + softmax(T) ==========
        with tc.tile_pool(name="ph1", bufs=1) as ph1:
            with tc.tile_pool(name="gps", bufs=1, space="PSUM") as gps:
                # G[t,j] = sum_m qT[m,t] kT[m,j], accumulated over m-chunks
                g_ps = gps.tile([65, 65], F32, name="g_ps", tag="g_ps")
                for i in range(NMC):
                    nc.tensor.matmul(g_ps[:], qt[:, 65 * i:65 * (i + 1)],
                                     kt[:, 65 * i:65 * (i + 1)],
                                     start=(i == 0), stop=(i == NMC - 1))
                g_sb = ph1.tile([65, 65], BF16, name="g_sb", tag="g_sb")
                nc.scalar.copy(g_sb[:], g_ps[:])

                # Pp = G^T Wq~^T : Pp[j,c] = sum_t G[t,j] wqe[t,c]
                pp_ps = gps.tile([65, 512], F32, name="pp_ps", tag="pp_ps")
                nc.tensor.matmul(pp_ps[:], g_sb[:], wqe, start=True,
                                 stop=True)
                pp_sb = ph1.tile([65, 512], BF16, name="pp_sb", tag="pp_sb")
                nc.scalar.copy(pp_sb[:], pp_ps[:])

            ET = [ph1.tile([128, 512], BF16, name=f"ET{dc}", tag=f"ET{dc}")
                  for dc in range(NCH)]
            with tc.tile_pool(name="stps", bufs=1, space="PSUM") as stps:
                # ST[d,c] = sum_j wke[j,d] Pp[j,c]  (4 d-chunks)
                st_ps = [stps.tile([128, 512], F32, name=f"st{dc}",
                                   tag=f"st{dc}") for dc in range(NCH)]
                # rowsums land transposed: column 4*dc+j of rsp_ps, all
                # single-shot matmuls (no in-bank accumulation groups)
                rsp_ps = stps.tile([128, 16], F32, name="rsp_ps",
                                   tag="rsp_ps")
                for dc in range(NCH):
                    nc.tensor.matmul(st_ps[dc][:],
                                     wke[:, 128 * dc:128 * (dc + 1)],
                                     pp_sb[:], start=True, stop=True)
                    nc.scalar.activation(ET[dc][:], st_ps[dc][:], AF.Exp,
                                         bias=0.0, scale=INV_SCALE)
                    for j in range(NCH):
                        nc.tensor.matmul(
                            rsp_ps[:, 4 * dc + j:4 * dc + j + 1],
                            ET[dc][:, 128 * j:128 * (j + 1)],
                            ones128[:], start=True, stop=True)
                rsum = ph1.tile([128, 4], F32, name="rsum", tag="rsum")
                nc.vector.reduce_sum(
                    rsum[:], rsp_ps.rearrange("p (dc j) -> p j dc", j=4),
                    axis=AX.X)
                recip_p = small.tile([128, 4], F32, name="recip_p",
                                     tag="recip_p")
                nc.vector.reciprocal(recip_p[:], rsum[:])

            with tc.tile_pool(name="hps", bufs=1, space="PSUM") as hps:
                # HTu = Wv~^T E^T : 4 accumulating matmuls
                htu_ps = hps.tile([65, 512], F32, name="htu_ps",
                                  tag="htu_ps")
                for dc in range(NCH):
                    nc.tensor.matmul(
                        htu_ps[:],
                        wpack[:, WVT0 + 65 * dc:WVT0 + 65 * (dc + 1)],
                        ET[dc][:], start=(dc == 0), stop=(dc == NCH - 1))
                htu_sb = ph1.tile([65, 512], BF16, name="htu_sb",
                                  tag="htu_sb")
                nc.scalar.copy(htu_sb[:], htu_ps[:])

                # H_n1[c, t] = HTu^T * recip (transpose + per-part scale)
                # (66-col stride: PSUM writes must be 4B aligned)
                tp_ps = hps.tile([128, 264], BF16, name="tp_ps", tag="tp_ps")
                h_n1 = ph1.tile([128, 260], BF16, name="h_n1", tag="h_n1")
                for j in range(NCH):
                    nc.tensor.transpose(tp_ps[:, 66 * j:66 * j + 65],
                                        htu_sb[:, 128 * j:128 * (j + 1)],
                                        identb[0:65, 0:65])
                    nc.scalar.activation(h_n1[:, 65 * j:65 * (j + 1)],
                                         tp_ps[:, 66 * j:66 * j + 65],
                                         AF.Copy, bias=0.0,
                                         scale=recip_p[:, j:j + 1])
                # M[t,t'] = sum_c Hn1[c,t] Hn1[c,t']
                # (full-bank tile: accumulation groups must own their bank —
                # any start=True matmul clears has_written for the whole bank)
                m_ps = hps.tile([65, 512], F32, name="m_ps", tag="m_ps")
                for j in range(NCH):
                    nc.tensor.matmul(m_ps[:, 0:65],
                                     h_n1[:, 65 * j:65 * (j + 1)],
                                     h_n1[:, 65 * j:65 * (j + 1)],
                                     start=(j == 0), stop=(j == NCH - 1))
                m_sb = small.tile([65, 65], BF16, name="m_sb", tag="m_sb")
                nc.vector.tensor_copy(m_sb[:], m_ps[:, 0:65])
                # HTn = transpose back: [65, 512]
                btp_ps = hps.tile([65, 512], BF16, name="btp_ps",
                                  tag="btp_ps")
                for j in range(NCH):
                    nc.tensor.transpose(btp_ps[:, 128 * j:128 * (j + 1)],
                                        h_n1[:, 65 * j:65 * (j + 1)],
                                        identb[:, 0:128])
                nc.scalar.copy(HTn[:], btp_ps[:])
            # u = HTn @ 1 : row sums [65, 1]
            u_sb = small.tile([65, 1], F32, name="u_sb", tag="u_sb")
            nc.vector.reduce_sum(u_sb[:], HTn[:], axis=AX.X)

            if dbg is not None:
                nc.sync.dma_start(dbg["g"][:], g_sb[:])
                for dc in range(NCH):
                    nc.sync.dma_start(dbg[f"ET{dc}"][:], ET[dc][:])
                nc.sync.dma_start(dbg["HTn"][:], HTn[:])
                nc.sync.dma_start(dbg["m"][:], m_sb[:])

        # ============ phase 2: factored BN stats -> AllReduce ===========
        # (emitted before X so the PE prioritizes unblocking the AR)
        with (
            tc.tile_pool(name="ph2", bufs=1) as ph2,
            tc.tile_pool(name="ssps", bufs=1, space="PSUM") as ssps,
            tc.tile_pool(name="zps", bufs=2, space="PSUM") as zps,
            tc.tile_pool(name="cdram", bufs=1, space="DRAM") as cdram,
            tc.tile_pool(name="xps", bufs=3, space="PSUM") as xps,
            tc.tile_pool(name="stp", bufs=1) as stp,
        ):
            # stat matmuls are all single-shot writes to distinct columns of
            # one bank (no PSUM accumulation groups — a start=True clears
            # has_written bank-wide); the r-reduction happens on the DVE.
            ss_ps = ssps.tile([128, 36], F32, name="ss_ps", tag="ss_ps")
            # sum[a] = sum_t W[t,a] u[t]
            for cc in range(NCH):
                nc.tensor.matmul(ss_ps[:, cc:cc + 1],
                                 Wf[:, 128 * cc:128 * (cc + 1)],
                                 u_sb[:], start=True, stop=True)
            # P[t, 512r + a] = v_r[t,a] * (M v_r)[t,a]; r-pairs share one
            # [65,1024] psum tile so each DVE multiply covers two r at once
            # (fewer DVE drains + semaphore hops on the AR critical path)
            P = ph2.tile([65, HW], BF16, name="P", tag="P")
            for g in range(NS // 2):
                z_ps = zps.tile([65, 1024], F32, name="z_ps", tag="z_ps")
                for h in range(2):
                    r = 2 * g + h
                    nc.tensor.matmul(z_ps[:, 512 * h:512 * (h + 1)],
                                     m_sb[:], vt[:, 512 * r:512 * (r + 1)],
                                     start=True, stop=True)
                nc.vector.tensor_mul(
                    P[:, 1024 * g:1024 * (g + 1)],
                    vt[:, 1024 * g:1024 * (g + 1)], z_ps[:])
            # sumsq part (cc, r) -> column 4 + 8*cc + r
            for r in range(NS):
                for cc in range(NCH):
                    nc.tensor.matmul(
                        ss_ps[:, 4 + 8 * cc + r:5 + 8 * cc + r],
                        P[:, 512 * r + 128 * cc:512 * r + 128 * (cc + 1)],
                        ones65[:], start=True, stop=True)
            red = stp.tile([128, 8], F32, name="red", tag="red")
            nc.vector.tensor_copy(red[:, 0:4], ss_ps[:, 0:4])
            nc.vector.reduce_sum(
                red[:, 4:8],
                ss_ps[:, 4:36].rearrange("p (cc r) -> p cc r", r=8),
                axis=AX.X)

            cin = cdram.tile([128, 8], F32, name="cin", tag="cin")
            cout = cdram.tile([128, 8], F32, name="cout", tag="cout")
            nc.gpsimd.dma_start(cin[:], red[:])
            nc.gpsimd.collective_compute(
                "AllReduce", ALU.add, replica_groups=RG,
                ins=[cin.opt()], outs=[cout.opt()])
            ar = stp.tile([128, 8], F32, name="ar", tag="ar")
            nc.sync.dma_start(ar[:], cout[:])

            # ======== phase 3: X = v_r^T HTn (overlaps the AllReduce) ====
            for cc in range(NCH):
                for r in range(NS):
                    xt = xps.tile([128, 512], F32, name="xt", tag="xt")
                    nc.tensor.matmul(
                        xt[:],
                        vt[:, 512 * r + 128 * cc:512 * r + 128 * (cc + 1)],
                        HTn[:], start=True, stop=True)
                    xsl = slice(512 * r, 512 * (r + 1))
                    if r % 2 == 0:
                        nc.scalar.copy(X[cc][:, xsl], xt[:])
                    else:
                        nc.vector.tensor_copy(X[cc][:, xsl], xt[:])

            # preload ACT tables while the AllReduce is in flight; the
            # read of `red` pins these after the stats trigger so their
            # table loads cannot be hoisted into the score chain
            dummy = stp.tile([128, 1], F32, name="dummy", tag="dummy")
            nc.scalar.activation(dummy[:], red[:, 0:1], AF.Lrelu,
                                 bias=0.0, scale=1.0, alpha=SLOPE)
            nc.scalar.activation(dummy[:], red[:, 0:1], AF.Sqrt,
                                 bias=epsb[:, 0:1])

            # BN affine params
            sd = stp.tile([128, 4], F32, name="sd", tag="sd")
            rstd = stp.tile([128, 4], F32, name="rstd", tag="rstd")
            tmp = stp.tile([128, 4], F32, name="tmpb", tag="tmpb")
            mv = stp.tile([128, 8], F32, name="mv", tag="mv")
            nc.vector.tensor_scalar_mul(mv[:], ar[:], INV_BHW)
            mean = mv[:, 0:4]
            var = mv[:, 4:8]
            nc.vector.tensor_mul(tmp[:], mean, mean)
            nc.vector.tensor_sub(var, var, tmp[:])
            nc.scalar.activation(sd[:], var, AF.Sqrt, bias=epsb[:, 0:1])
            nc.vector.reciprocal(rstd[:], sd[:])
            nc.vector.tensor_mul(alpha[:], bngsb, rstd[:])
            nc.vector.tensor_mul(tmp[:], mean, alpha[:])
            nc.vector.tensor_sub(beta[:], bnbsb, tmp[:])

            if dbg is not None:
                nc.sync.dma_start(dbg["P"][:], P[:])
                nc.sync.dma_start(dbg["red"][:], red[:])
                for cc in range(NCH):
                    nc.sync.dma_start(dbg[f"X{cc}"][:], X[cc][:])
                    nc.sync.dma_start(dbg[f"ar{cc}"][:, 0:1],
                                      ar[:, cc:cc + 1])
                    nc.sync.dma_start(dbg[f"ar{cc}"][:, 1:2],
                                      ar[:, 4 + cc:5 + cc])

        # ============ phase 4: BN+leaky -> w1 -> leaky -> w2 -> y =======
        with (
            tc.tile_pool(name="y2", bufs=2) as y2p,
            tc.tile_pool(name="bnp", bufs=2) as bnp,
            tc.tile_pool(name="outb", bufs=1) as outp,
            tc.tile_pool(name="wps", bufs=3, space="PSUM") as wps,
            tc.tile_pool(name="w2ps", bufs=2, space="PSUM") as w2ps,
        ):
            osb = outp.tile([64, HW], F32, name="osb", tag="osb")

            def bn_act(ms):
                # first slice on ACT (idle right after the AR): single-op
                # Lrelu unblocks the first w1 matmuls ~0.7us sooner and runs
                # in parallel with the DVE's alpha/beta + bn(1)
                ssl = slice(512 * ms, 512 * (ms + 1))
                for cc in range(NCH):
                    nc.scalar.activation(X[cc][:, ssl], X[cc][:, ssl],
                                         AF.Lrelu, bias=beta[:, cc:cc + 1],
                                         scale=alpha[:, cc:cc + 1],
                                         alpha=SLOPE)

            def bn(ms, width=1):
                # on DVE (ACT is saturated by y2): t = a*X+b; X = max(t, .01t)
                ssl = slice(512 * ms, 512 * (ms + width))
                for cc in range(NCH):
                    t = bnp.tile([128, 512 * width], BF16, name="bnt",
                                 tag="bnt", bufs=2)
                    nc.vector.tensor_scalar(
                        out=t[:], in0=X[cc][:, ssl],
                        scalar1=alpha[:, cc:cc + 1],
                        scalar2=beta[:, cc:cc + 1],
                        op0=ALU.mult, op1=ALU.add)
                    nc.vector.scalar_tensor_tensor(
                        out=X[cc][:, ssl], in0=t[:], scalar=SLOPE,
                        in1=t[:], op0=ALU.mult, op1=ALU.max)

            def tail_pair(mp):
                # two ms-slices (2mp, 2mp+1) share [128,1024] psum tiles so
                # the Lrelu activations run 1024 wide (amortize ACT overhead)
                y2t = []
                for oc in range(NCH):
                    wp = wps.tile([128, 1024], F32, name="wp", tag="wp")
                    for h in range(2):
                        ssl = slice(512 * (2 * mp + h), 512 * (2 * mp + h + 1))
                        for cc in range(NCH):
                            nc.tensor.matmul(
                                wp[:, 512 * h:512 * (h + 1)],
                                wpack[:, W1T0 + 512 * cc + 128 * oc:
                                      W1T0 + 512 * cc + 128 * (oc + 1)],
                                X[cc][:, ssl], start=(cc == 0), stop=(cc == 3))
                    yt = y2p.tile([128, 1024], BF16, name=f"y2_{oc}",
                                  tag=f"y2_{oc}")
                    nc.scalar.activation(yt[:], wp[:], AF.Lrelu,
                                         bias=b1sb[:, oc:oc + 1],
                                         scale=1.0, alpha=SLOPE)
                    y2t.append(yt)
                for h in range(2):
                    ssl = slice(512 * (2 * mp + h), 512 * (2 * mp + h + 1))
                    fp = w2ps.tile([64, 512], F32, name="fp", tag="fp")
                    for oc in range(NCH):
                        nc.tensor.matmul(
                            fp[:],
                            wpack[:, W2T0 + 64 * oc:W2T0 + 64 * (oc + 1)],
                            y2t[oc][:, 512 * h:512 * (h + 1)],
                            start=(oc == 0), stop=(oc == 3))
                    nc.vector.tensor_scalar_add(osb[:, ssl], fp[:],
                                                b2sb[:, 0:1])
                    if h == 0:
                        nc.sync.dma_start(d["y"][:, ssl], osb[:, ssl])
                    else:
                        nc.scalar.dma_start(d["y"][:, ssl], osb[:, ssl])

            def tail_one(ms):
                ssl = slice(512 * ms, 512 * (ms + 1))
                y2t = []
                for oc in range(NCH):
                    wp = wps.tile([128, 1024], F32, name="wp", tag="wp")
                    for cc in range(NCH):
                        nc.tensor.matmul(
                            wp[:, 0:512],
                            wpack[:, W1T0 + 512 * cc + 128 * oc:
                                  W1T0 + 512 * cc + 128 * (oc + 1)],
                            X[cc][:, ssl], start=(cc == 0), stop=(cc == 3))
                    yt = y2p.tile([128, 1024], BF16, name=f"y2_{oc}",
                                  tag=f"y2_{oc}")
                    nc.scalar.activation(yt[:, 0:512], wp[:, 0:512],
                                         AF.Lrelu, bias=b1sb[:, oc:oc + 1],
                                         scale=1.0, alpha=SLOPE)
                    y2t.append(yt)
                fp = w2ps.tile([64, 512], F32, name="fp", tag="fp")
                for oc in range(NCH):
                    nc.tensor.matmul(
                        fp[:], wpack[:, W2T0 + 64 * oc:W2T0 + 64 * (oc + 1)],
                        y2t[oc][:, 0:512], start=(oc == 0), stop=(oc == 3))
                nc.vector.tensor_scalar_add(osb[:, ssl], fp[:], b2sb[:, 0:1])
                nc.sync.dma_start(d["y"][:, ssl], osb[:, ssl])

            bn(0)
            bn(1)
            for mp in range(3):
                tail_pair(mp)
                bn(2 * mp + 2, width=2)
            tail_one(6)
            tail_one(7)


_NC_CACHE = {}


def _build(debug=False):
    key = ("dbg" if debug else "nc")
    if key in _NC_CACHE:
        return _NC_CACHE[key]
    nc = bacc.Bacc(trn_type="TRN2", target_bir_lowering=False, debug=False,
                   enable_asserts=False, num_devices=8)
    d = {}
    d["qt"] = nc.dram_tensor("qt", (128, 2080), BF16, kind="ExternalInput").ap()
    d["kt"] = nc.dram_tensor("kt", (128, 2080), BF16, kind="ExternalInput").ap()
    d["vt"] = nc.dram_tensor("vt", (65, HW), BF16, kind="ExternalInput").ap()
    d["wpack"] = nc.dram_tensor("wpack", (128, WCOLS), BF16,
                                kind="ExternalInput").ap()
    d["spack"] = nc.dram_tensor("spack", (128, 16), F32,
                                kind="ExternalInput").ap()
    d["y"] = nc.dram_tensor("y", (64, HW), F32, kind="ExternalOutput").ap()

    dbg = None
    if debug:
        dbg = {}
        dbg["g"] = nc.dram_tensor("dbg_g", (65, 65), BF16, kind="ExternalOutput").ap()
        dbg["HTn"] = nc.dram_tensor("dbg_HTn", (65, 512), BF16, kind="ExternalOutput").ap()
        dbg["m"] = nc.dram_tensor("dbg_m", (65, 65), BF16, kind="ExternalOutput").ap()
        dbg["P"] = nc.dram_tensor("dbg_P", (65, HW), BF16, kind="ExternalOutput").ap()
        dbg["red"] = nc.dram_tensor("dbg_red", (128, 8), F32, kind="ExternalOutput").ap()
        for cc in range(NCH):
            dbg[f"ET{cc}"] = nc.dram_tensor(f"dbg_ET{cc}", (128, 512), BF16, kind="ExternalOutput").ap()
            dbg[f"X{cc}"] = nc.dram_tensor(f"dbg_X{cc}", (128, HW), BF16, kind="ExternalOutput").ap()
            dbg[f"ar{cc}"] = nc.dram_tensor(f"dbg_ar{cc}", (128, 2), F32, kind="ExternalOutput").ap()
    with tile.TileContext(nc) as tc:
        _body(tc, nc, d, dbg)
    nc.compile()
    _NC_CACHE[key] = nc
    return nc


def _prep(q, k, v, wq, bq, wk, bk, wv, bv, bn_g, bn_b, w1, b1, w2, b2):
    f = np.float32
    bf = ml_dtypes.bfloat16
    wpack = np.zeros((128, WCOLS), f)
    wpack[0:65, WQE0:WQE0 + 512] = np.concatenate([wq.T, bq[None, :]], axis=0)
    wpack[0:65, WKE0:WKE0 + 512] = np.concatenate([wk.T, bk[None, :]], axis=0)
    wve = np.concatenate([wv.T, bv[None, :]], axis=0)  # [65, 512] = Wv^T aug
    for j in range(NCH):
        # wvT chunk j: [128d, 65t] = Wv~[128j:128(j+1), :] = wve.T slice
        wpack[:, WVT0 + 65 * j:WVT0 + 65 * (j + 1)] = \
            wve[:, 128 * j:128 * (j + 1)].T
    w1t = w1.T.astype(f)
    for cc in range(NCH):
        wpack[:, W1T0 + 512 * cc:W1T0 + 512 * (cc + 1)] = \
            w1t[128 * cc:128 * (cc + 1), :]
    w2t = w2.T.astype(f)
    for oc in range(NCH):
        wpack[:, W2T0 + 64 * oc:W2T0 + 64 * (oc + 1)] = \
            w2t[128 * oc:128 * (oc + 1), :]
    wpack[:, IDB0:IDB0 + 128] = np.eye(128, dtype=f)
    spack = np.zeros((128, 16), f)
    spack[:, 0:4] = b1.reshape(4, 128).T
    spack[:, 4:8] = bn_g.reshape(4, 128).T
    spack[:, 8:12] = bn_b.reshape(4, 128).T
    spack[0:64, 12] = b2

    shared = {"wpack": wpack.astype(bf), "spack": spack}
    in_maps = []
    ones_col = np.ones((HW, 1), f)
    for b in range(B):
        m = dict(shared)
        for name, x in (("qt", q[b]), ("kt", k[b])):
            xt = np.concatenate([x.reshape(C, HW).T, ones_col], axis=1)
            m[name] = np.ascontiguousarray(
                xt.reshape(NMC, 128, 65).transpose(1, 0, 2).reshape(128, 2080)
            ).astype(bf)
        vtn = np.concatenate(
            [v[b].reshape(C, HW), np.ones((1, HW), f)], axis=0)
        # permute to r-major blocks: vt[t, 512r + a] = v~[t, 8a + r], so
        # every device-side slice of v is contiguous (stride-8 access
        # patterns run the PE/DVE at 1/3-1/5 rate)
        m["vt"] = np.ascontiguousarray(
            vtn.reshape(65, 512, 8).transpose(0, 2, 1).reshape(65, HW)
        ).astype(bf)
        in_maps.append(m)
    return in_maps


def _run(q, k, v, wq, bq, wk, bk, wv, bv, bn_g, bn_b, w1, b1, w2, b2,
         trace=False, tmpdir=None, debug=False):
    nc = _build(debug)
    in_maps = _prep(q, k, v, wq, bq, wk, bk, wv, bv, bn_g, bn_b, w1, b1,
                    w2, b2)
    res = bass_utils.run_bass_kernel_spmd(
        nc, in_maps, core_ids=list(range(8)), trace=trace, tmpdir=tmpdir)
    out = np.stack([res.results[b]["y"].reshape(C, 64, 64) for b in range(B)])
    return out.astype(np.float32), res


def kernel(q, k, v, wq, bq, wk, bk, wv, bv, bn_g, bn_b, w1, b1, w2, b2):
    out, _ = _run(q, k, v, wq, bq, wk, bk, wv, bv, bn_g, bn_b, w1, b1, w2, b2)
    return out
